# revision 1
# baseline (speedup 1.0000x reference)
"""ALIGNNConv edge-gate kernel for 8 TRN2 NeuronCores.

reference math:
    row, col = edge_index
    x = concat([nf[row], nf[col], ef], -1)        # [E, 384]
    gate = sigmoid(x @ W + b)                     # [E, 128]
    out = ef * gate

Strategy (per the sharding hint): shard edges across 8 cores, replicate
node data. Per core the kernel decomposes x@W as
    nf[row]@W1 + nf[col]@W2 + ef@W3
and precomputes P_row = nf@W1, P_col = nf@W2 + b ONCE into bf16 DRAM
tables (projection phase). The edge phase gathers 128-float rows from the
two tables with the SWDGE ucode dma_gather (int16 indices), adds them into
the ef@W3 PSUM accumulation via identity matmuls, applies sigmoid on the
scalar engine, and multiplies by ef on the vector engine.

dma_gather limits: indices are int16 (< 32768) and num_idxs <= 1024 per
call. The node tables have 51200 (padded) rows, so each table is used as
two 25600-row halves, and the host sorts each core's edges into four
sections by (row-half, col-half). Each section is padded to a fixed
21 x 1024 slots so the compiled program is data-independent; each group of
1024 edges then gathers from a compile-time-known table half with all
indices valid.

Host-side work is layout-only: dtype casts (f32->bf16, int64->int16),
transposes, the bucket permutation (undone on output), and the final
bf16->f32 upcast.
"""

import numpy as np
import ml_dtypes

BF16 = ml_dtypes.bfloat16

N_NODES = 50000
N_EDGES = 640000
D = 128
N_CORES = 8


class Cfg:
    def __init__(self, nodes_pad, nf_stage, sec_groups, k_idx):
        assert nodes_pad % nf_stage == 0 and nf_stage % D == 0
        assert nodes_pad % 2 == 0
        assert (nodes_pad // 2) % nf_stage == 0, "half must divide into stages"
        assert nf_stage % 512 == 0 or nf_stage == 512, "psum batch of 4x128"

        self.nodes_pad = nodes_pad
        self.half = nodes_pad // 2          # rows per table half (< 32768)
        assert self.half < 32768
        self.nf_stage = nf_stage
        if isinstance(sec_groups, int):
            sec_groups = [sec_groups] * 4
        self.sec_groups = tuple(sec_groups)  # gather groups per section
        self.groups = sum(self.sec_groups)
        self.k_idx = k_idx                  # 128-edge subtiles per group
        self.group_e = D * k_idx            # edges per group (num_idxs <= 1024)
        assert self.group_e <= 1024
        self.sec_cap = tuple(g * self.group_e for g in self.sec_groups)
        self.sec_off = tuple(
            sum(self.sec_cap[:s]) for s in range(5)
        )  # slot offset per section (5th = e_slots)
        self.e_slots = self.sec_off[4]
        # group index -> section
        self.group_sec = []
        for sidx, ng in enumerate(self.sec_groups):
            self.group_sec.extend([sidx] * ng)


FULL_CFG = Cfg(nodes_pad=51200, nf_stage=5120, sec_groups=[21, 20, 20, 19], k_idx=8)
E_CORE = N_EDGES // N_CORES


def build_nc(cfg: Cfg, repeat: int = 1, variant: str = "full", gbufs: int = 6, spq: bool = True, pbufs: int = 2, ebufs: int = 2, tbufs: int = 2):
    """repeat > 1 wraps the whole body in a For_i loop for benchmarking:
    the body is idempotent, so N executions inside one NEFF let wall-clock
    slope measurements cancel the (huge, noisy) axon dispatch overhead.

    variant: benchmark-only ablations (results garbage unless "full"):
      full       - the real kernel
      noproj     - projection phase off (tables get a token init write)
      nogather   - no dma_gather; logits = ef@W3 only
      nocompute  - no matmul/sigmoid/mul; out streams ef_t back out
      gatheronly - idx loads + gathers + out only, projection off
    """
    assert variant in ("full", "noproj", "nogather", "nocompute", "gatheronly", "projonly")
    no_proj = variant in ("noproj", "gatheronly")
    no_gather = variant == "nogather"
    no_compute = variant in ("nocompute", "gatheronly")
    no_edgedma = variant == "gatheronly"
    import concourse.bass as bass
    import concourse.mybir as mybir
    from concourse import bacc
    from concourse import library_config
    from concourse.tile import TileContext
    from concourse.tile_rust import add_dep_helper

    f32 = mybir.dt.float32
    bf16 = mybir.dt.bfloat16
    i16 = mybir.dt.int16

    H = cfg.half
    nc = bacc.Bacc(
        "TRN2", target_bir_lowering=False, debug=False, num_swdge_queues=4
    )

    nft = nc.declare_dram_parameter("nft", [D, cfg.nodes_pad], bf16, isOutput=False)
    w = nc.declare_dram_parameter("w", [3 * D, D], bf16, isOutput=False)
    bb = nc.declare_dram_parameter("bb", [D, D], bf16, isOutput=False)
    ident = nc.declare_dram_parameter("ident", [D, D], bf16, isOutput=False)
    n_iw = cfg.group_e // 16
    idxr = nc.declare_dram_parameter(
        "idxr", [D, cfg.groups * n_iw], i16, isOutput=False
    )
    idxc = nc.declare_dram_parameter(
        "idxc", [D, cfg.groups * n_iw], i16, isOutput=False
    )
    eft = nc.declare_dram_parameter("eft", [D, cfg.e_slots], bf16, isOutput=False)
    outp = nc.declare_dram_parameter(
        "out", [cfg.groups, D, cfg.group_e], bf16, isOutput=True
    )

    with TileContext(nc) as tc:
        with (
            tc.tile_pool(name="dram", bufs=1, space="DRAM") as dpool,
            tc.tile_pool(name="const", bufs=1) as cpool,
            tc.tile_pool(name="nfst", bufs=2) as nfpool,
            tc.tile_pool(name="pps", bufs=pbufs, space="PSUM") as pps,
            tc.tile_pool(name="psb", bufs=4) as psb,
            tc.tile_pool(name="gat", bufs=gbufs) as gpool,
            tc.tile_pool(name="edg", bufs=3) as epool,
            tc.tile_pool(name="gsb", bufs=4) as gspool,
            tc.tile_pool(name="eps", bufs=ebufs, space="PSUM") as eps,
            tc.tile_pool(name="gtp", bufs=tbufs, space="PSUM") as gtps,
        ):
            nc.gpsimd.load_library(library_config.mlp)

            prow_h = [
                dpool.tile([H, D], bf16, name=f"prow{h}") for h in range(2)
            ]
            pcol_h = [
                dpool.tile([H, D], bf16, name=f"pcol{h}") for h in range(2)
            ]

            def emit_body():
                emit_consts()
                if variant == "projonly":
                    emit_projection()
                    return
                if not no_proj:
                    emit_projection()
                else:
                    # token init so the tables have a writer for dep tracking
                    for h in range(2):
                        nc.sync.dma_start(out=prow_h[h][0:D, :], in_=bb[:, :])
                        nc.sync.dma_start(out=pcol_h[h][0:D, :], in_=bb[:, :])
                emit_edges()

            w1 = cpool.tile([D, D], bf16, name="w1")
            w2 = cpool.tile([D, D], bf16, name="w2")
            w3 = cpool.tile([D, D], bf16, name="w3")
            bbt = cpool.tile([D, D], bf16, name="bbt")
            bbt4 = cpool.tile([D, 4 * D], bf16, name="bbt4")
            idt = cpool.tile([D, D], bf16, name="idt")

            def emit_consts():
                nc.sync.dma_start(out=w1[:], in_=w[0:D, :])
                nc.sync.dma_start(out=w2[:], in_=w[D : 2 * D, :])
                nc.sync.dma_start(out=w3[:], in_=w[2 * D : 3 * D, :])
                nc.sync.dma_start(out=bbt[:], in_=bb[:, :])
                for r4 in range(4):
                    nc.sync.dma_start(
                        out=bbt4[:, r4 * D : (r4 + 1) * D], in_=bb[:, :]
                    )
                nc.sync.dma_start(out=idt[:], in_=ident[:, :])

            # ---- projection phase: P_row = nf@W1, P_col = nf@W2 + b ----
            def emit_projection():
                B = 4  # 128-node chunks per PSUM bank (512 f32 = 1 bank)
                stages_per_half = cfg.half // cfg.nf_stage
                for s in range(cfg.nodes_pad // cfg.nf_stage):
                    half = s // stages_per_half
                    nfs = nfpool.tile([D, cfg.nf_stage], bf16, name="nfs")
                    nc.sync.dma_start(
                        out=nfs[:],
                        in_=nft[:, s * cfg.nf_stage : (s + 1) * cfg.nf_stage],
                    )
                    prs = psb.tile([D, cfg.nf_stage], bf16, name="prs")
                    pcs = psb.tile([D, cfg.nf_stage], bf16, name="pcs")
                    for jb in range(cfg.nf_stage // (B * D)):
                        p1 = pps.tile([D, B * D], f32, name="p1")
                        p2 = pps.tile([D, B * D], f32, name="p2")
                        for j4 in range(B):
                            j = jb * B + j4
                            lhsT = nfs[:, j * D : (j + 1) * D]
                            psl = slice(j4 * D, (j4 + 1) * D)
                            nc.tensor.matmul(
                                p1[:, psl], lhsT, w1[:], start=True, stop=True
                            )
                            nc.tensor.matmul(
                                p2[:, psl], lhsT, w2[:], start=True, stop=True
                            )
                        bsl = slice(jb * B * D, (jb + 1) * B * D)
                        if jb % 2 == 0:
                            nc.scalar.copy(prs[:, bsl], p1[:])
                        else:
                            nc.vector.tensor_copy(prs[:, bsl], p1[:])
                        nc.vector.tensor_add(pcs[:, bsl], p2[:], bbt4[:])
                    lo = s * cfg.nf_stage - half * cfg.half
                    hi = lo + cfg.nf_stage
                    nc.sync.dma_start(
                        out=prow_h[half][lo:hi, :].rearrange(
                            "(j p) f -> p j f", p=D
                        ),
                        in_=prs[:].rearrange("p (j f) -> p j f", f=D),
                    )
                    nc.sync.dma_start(
                        out=pcol_h[half][lo:hi, :].rearrange(
                            "(j p) f -> p j f", p=D
                        ),
                        in_=pcs[:].rearrange("p (j f) -> p j f", f=D),
                    )

            # ---- edge phase ----
            def emit_edges():
                NI = cfg.group_e
                niw = NI // 16
                ira = cpool.tile([D, cfg.groups * niw], i16, name="ira")
                ica = cpool.tile([D, cfg.groups * niw], i16, name="ica")
                nc.sync.dma_start(out=ira[:], in_=idxr[:, :])
                nc.sync.dma_start(out=ica[:], in_=idxc[:, :])
                prev_gather = None
                for g in range(cfg.groups):
                    sec = cfg.group_sec[g]
                    rh, ch = sec >> 1, sec & 1
                    gr = gpool.tile([D, cfg.k_idx, D], bf16, name="gr")
                    gc = gpool.tile([D, cfg.k_idx, D], bf16, name="gc")
                    if not no_gather:
                        g1 = nc.gpsimd.dma_gather(
                            gr[:],
                            prow_h[rh][:, :],
                            ira[:, g * niw : (g + 1) * niw],
                            NI, NI, D,
                            queue_num=(2 * g) % 4,
                            single_packet=spq,
                        )
                        g2 = nc.gpsimd.dma_gather(
                            gc[:],
                            pcol_h[ch][:, :],
                            ica[:, g * niw : (g + 1) * niw],
                            NI, NI, D,
                            queue_num=(2 * g + 1) % 4,
                            single_packet=spq,
                        )
                        # keep Pool-engine issue order = program order so
                        # Tile's round-robin DMASW sem lanes stay aligned
                        # with the 4-queue cycle (sems are queue-locked)
                        if prev_gather is not None:
                            add_dep_helper(
                                g1.ins, prev_gather.ins, sync=False,
                                reason="swdge lane/queue alignment",
                            )
                        add_dep_helper(
                            g2.ins, g1.ins, sync=False,
                            reason="swdge lane/queue alignment",
                        )
                        prev_gather = g2
                    eft_t = epool.tile([D, NI], bf16, name="eft_t")
                    if not no_edgedma:
                        nc.sync.dma_start(
                            out=eft_t[:], in_=eft[:, g * NI : (g + 1) * NI]
                        )
                    o_t = epool.tile([D, NI], bf16, name="o_t")
                    if not no_compute:
                        EB = 4  # subtiles per PSUM bank batch
                        for kb in range(cfg.k_idx // EB):
                            bsl = slice(kb * EB * D, (kb + 1) * EB * D)
                            ps4 = eps.tile([D, EB * D], f32, name="ps4")
                            for k4 in range(EB):
                                k = kb * EB + k4
                                sl = slice(k * D, (k + 1) * D)
                                psl = slice(k4 * D, (k4 + 1) * D)
                                nc.tensor.matmul(
                                    ps4[:, psl],
                                    eft_t[:, sl],
                                    w3[:],
                                    start=True,
                                    stop=no_gather,
                                )
                                if not no_gather:
                                    nc.tensor.matmul(
                                        ps4[:, psl], idt[:], gr[:, k, :],
                                        start=False, stop=False,
                                    )
                                    nc.tensor.matmul(
                                        ps4[:, psl], idt[:], gc[:, k, :],
                                        start=False, stop=True,
                                    )
                            gate4 = gspool.tile([D, EB * D], bf16, name="gate4")
                            nc.scalar.activation(
                                gate4[:], ps4[:], mybir.ActivationFunctionType.Sigmoid
                            )
                            gt4 = gtps.tile([D, EB * D], bf16, name="gt4")
                            for k4 in range(EB):
                                psl = slice(k4 * D, (k4 + 1) * D)
                                nc.tensor.transpose(
                                    gt4[:, psl], gate4[:, psl], idt[:]
                                )
                            nc.vector.tensor_mul(
                                o_t[:, bsl], eft_t[:, bsl], gt4[:]
                            )
                        nc.sync.dma_start(out=outp[g, :, :], in_=o_t[:])
                    elif not no_edgedma:
                        nc.sync.dma_start(out=outp[g, :, :], in_=eft_t[:])
                    else:
                        nc.sync.dma_start(
                            out=outp[g, :, :],
                            in_=gr[:].rearrange("p k d -> p (k d)"),
                        )

            if repeat > 1:
                with tc.For_i(0, repeat, 1):
                    emit_body()
            else:
                emit_body()

    nc.finalize()
    return nc


def bucketize(cfg: Cfg, er32, ec32):
    """Sort edges into 4 (row-half, col-half) sections with fixed capacity.

    Returns (perm, n_edges_per_section) where perm[slot] = original edge id
    or -1 for padding slots.
    """
    H = cfg.half
    sec = (er32 >= H).astype(np.int64) * 2 + (ec32 >= H)
    counts = np.bincount(sec, minlength=4)
    if np.any(counts > np.asarray(cfg.sec_cap)):
        raise ValueError(
            f"section sizes {counts} exceed capacity {cfg.sec_cap}; "
            f"node distribution too skewed for compiled bucket layout"
        )
    # row-sorted within each section: row-gathers become nearly
    # sequential HBM reads (col-gathers stay random)
    order = np.lexsort((er32, sec))
    perm = np.full(cfg.e_slots, -1, dtype=np.int64)
    off = 0
    for s in range(4):
        n = counts[s]
        perm[cfg.sec_off[s] : cfg.sec_off[s] + n] = order[off : off + n]
        off += n
    return perm, counts


def wrap_idx(cfg: Cfg, idx_slot):
    """[e_slots] int16 -> [128, groups * group_e//16] dma_gather index layout
    (16-partition wrap per group, replicated to 128 partitions, groups
    concatenated along the free dim so one DMA stages all of them)."""
    g, ni = cfg.groups, cfg.group_e
    a = idx_slot.reshape(g, ni // 16, 16).transpose(0, 2, 1)  # [g, 16, ni//16]
    a = np.tile(a, (1, 8, 1))                                 # [g, 128, ni//16]
    return np.ascontiguousarray(a.transpose(1, 0, 2).reshape(128, -1))


def make_in_maps(cfg: Cfg, node_features, edge_index, edge_features, W, b, n_cores):
    H = cfg.half
    nf = np.asarray(node_features, dtype=np.float32)
    nf_pad = np.zeros((cfg.nodes_pad, D), dtype=np.float32)
    nf_pad[: nf.shape[0]] = nf
    nft = np.ascontiguousarray(nf_pad.T.astype(BF16))

    w_bf = np.ascontiguousarray(np.asarray(W, dtype=np.float32).astype(BF16))
    b32 = np.asarray(b, dtype=np.float32)
    bb = np.ascontiguousarray(np.tile(b32.astype(BF16), (D, 1)))
    ident = np.eye(D, dtype=BF16)

    ei = np.asarray(edge_index)
    ef = np.asarray(edge_features, dtype=np.float32)

    e_core = ei.shape[1] // n_cores
    in_maps = []
    perms = []
    for i in range(n_cores):
        sl = slice(i * e_core, (i + 1) * e_core)
        er32 = ei[0, sl].astype(np.int32)
        ec32 = ei[1, sl].astype(np.int32)
        ef_bf = ef[sl].astype(BF16)

        perm, _ = bucketize(cfg, er32, ec32)
        filled = perm >= 0
        src = perm[filled]

        er_slot = np.zeros(cfg.e_slots, dtype=np.int32)
        ec_slot = np.zeros(cfg.e_slots, dtype=np.int32)
        er_slot[filled] = er32[src]
        ec_slot[filled] = ec32[src]
        # subtract the half offset per section (compile-time table half)
        sec_of_slot = np.repeat(np.arange(4), np.asarray(cfg.sec_cap))
        er_slot -= ((sec_of_slot >> 1) * H).astype(np.int32)
        ec_slot -= ((sec_of_slot & 1) * H).astype(np.int32)
        er_slot[~filled] = 0
        ec_slot[~filled] = 0
        assert er_slot.min() >= 0 and er_slot.max() < H
        assert ec_slot.min() >= 0 and ec_slot.max() < H

        ef_slot = np.zeros((cfg.e_slots, D), dtype=BF16)
        ef_slot[filled] = ef_bf[src]

        in_maps.append(
            {
                "nft": nft,
                "w": w_bf,
                "bb": bb,
                "ident": ident,
                "idxr": wrap_idx(cfg, er_slot.astype(np.int16)),
                "idxc": wrap_idx(cfg, ec_slot.astype(np.int16)),
                "eft": np.ascontiguousarray(ef_slot.T),
            }
        )
        perms.append(perm)
    return in_maps, perms


def unpack_out(cfg: Cfg, o, perm, e_core):
    """[groups, D, group_e] bf16 feat-major slot output -> [e_core, D] f32."""
    slots = (
        np.asarray(o)
        .reshape(cfg.groups, D, cfg.group_e)
        .transpose(0, 2, 1)
        .reshape(cfg.e_slots, D)
    ).astype(np.float32)
    res = np.empty((e_core, D), dtype=np.float32)
    filled = perm >= 0
    res[perm[filled]] = slots[filled]
    return res


_CACHE = {}


def derive_cfg(edge_index, n_cores):
    """Pick per-section group counts from the actual index distribution."""
    ei = np.asarray(edge_index)
    e_core = ei.shape[1] // n_cores
    H = FULL_CFG.half
    mx = np.zeros(4, dtype=np.int64)
    for i in range(n_cores):
        sl = slice(i * e_core, (i + 1) * e_core)
        sec = (ei[0, sl] >= H).astype(np.int64) * 2 + (ei[1, sl] >= H)
        mx = np.maximum(mx, np.bincount(sec, minlength=4))
    ni = FULL_CFG.group_e
    sg = [int(-(-int(c) // ni)) for c in mx]
    return Cfg(FULL_CFG.nodes_pad, FULL_CFG.nf_stage, sg, FULL_CFG.k_idx)


def kernel(node_features, edge_index, edge_features, W, b):
    from concourse.bass_utils import run_bass_kernel_spmd

    cfg = derive_cfg(edge_index, N_CORES)
    key = cfg.sec_groups
    if key not in _CACHE:
        _CACHE[key] = build_nc(cfg)
    nc = _CACHE[key]

    in_maps, perms = make_in_maps(
        cfg, node_features, edge_index, edge_features, W, b, N_CORES
    )
    res = run_bass_kernel_spmd(nc, in_maps, core_ids=list(range(N_CORES)))
    e_core = np.asarray(edge_index).shape[1] // N_CORES
    outs = [
        unpack_out(cfg, res.results[i]["out"], perms[i], e_core)
        for i in range(N_CORES)
    ]
    return np.concatenate(outs, axis=0)



# revision 2
# speedup vs baseline: 1.7089x; 1.7089x over previous
"""ALIGNNConv edge-gate kernel for 8 TRN2 NeuronCores — v5 (no projection,
edge-major gathers, feature-major compute).

reference math:
    row, col = edge_index
    x = concat([nf[row], nf[col], ef], -1)        # [E, 384]
    gate = sigmoid(x @ W + b)                     # [E, 128]
    out = ef * gate

The kernel is HBM-bandwidth bound (all 8 cores share the stacks), so v5
minimizes bytes: NO precomputed projection tables. The SWDGE dma_gather
pulls raw 256-byte node-feature rows (edge-major, the only gather mode
this hardware supports), the PE transposes each gathered 128-edge subtile
into PSUM, a copy moves it to SBUF feature-major, and each 512-edge PSUM
bank accumulates three stationary-weight matmuls
    psum = W1^T @ gr_t + W2^T @ gc_t + W3^T @ ef
followed by sigmoid(psum + b) on the scalar engine (per-partition bias)
and ef*gate on the vector engine. Per-core HBM traffic is ~89MB vs ~128MB
for the projection-table variant; the extra PE/ACT/DVE work hides under
the DMA (measured: full compute adds <30us over pure streams).

dma_gather limits (probed on HW): DRAM source, transpose=False only,
num_idxs <= 1024 per call, 256B-multiple rows. int16 indices, so the node
table is used as two 25600-row halves and the host sorts each core's
edges into four sections by (row-half, col-half), row-sorted within a
section so row gathers hit HBM nearly sequentially. Sections are padded
to a fixed capacity (multiple of 512) so the program is data-independent.

Host-side work is layout-only: dtype casts (f32->bf16, int64->int16),
transposes, the bucket permutation (undone on output), and the final
bf16->f32 upcast.
"""

import os as _os

import numpy as np
import ml_dtypes

BF16 = ml_dtypes.bfloat16

N_NODES = 50000
N_EDGES = 640000
D = 128
N_CORES = 8
NODES_PAD = 51200
H = NODES_PAD // 2  # 25600 rows per table half (< 32768 for int16)
GROUP_MAX = int(_os.environ.get("V5_GROUP_MAX", "1024"))
SCRATCH = int(_os.environ.get("V5_SCRATCH", "65536"))


def _section_groups(cap):
    gs = []
    while cap >= GROUP_MAX:
        gs.append(GROUP_MAX)
        cap -= GROUP_MAX
    if cap > 0:
        gs.append(cap)
    return gs


class Cfg:
    def __init__(self, sec_cap):
        assert len(sec_cap) == 4
        for c in sec_cap:
            assert c % 512 == 0 and c > 0
        self.sec_cap = tuple(int(c) for c in sec_cap)
        self.sec_off = tuple(sum(self.sec_cap[:s]) for s in range(5))
        self.e_slots = self.sec_off[4]
        self.groups = []
        for s in range(4):
            off = self.sec_off[s]
            for g in _section_groups(self.sec_cap[s]):
                self.groups.append((s, off, g))
                off += g


E_CORE = N_EDGES // N_CORES


def build_nc(cfg: Cfg, repeat: int = 1, variant: str = "full"):
    """repeat > 1 wraps the whole body in a For_i loop for benchmarking.

    variant: benchmark-only ablations (results garbage unless "full"):
      full       - the real kernel
      nogather   - no dma_gather; logits = ef@W3 only
      nocompute  - no matmul/sigmoid/mul; out streams ef back out
      gatheronly - idx loads + gathers + out writes only
    """
    assert variant in ("full", "nogather", "nocompute", "gatheronly")
    no_gather = variant in ("nogather", "nocompute")
    no_compute = variant in ("nocompute", "gatheronly")
    no_edgedma = variant == "gatheronly"
    import concourse.bass as bass
    import concourse.mybir as mybir
    from concourse import bacc
    from concourse import library_config
    from concourse.tile import TileContext
    from concourse.tile_rust import add_dep_helper

    f32 = mybir.dt.float32
    bf16 = mybir.dt.bfloat16
    i16 = mybir.dt.int16

    nc = bacc.Bacc(
        "TRN2",
        target_bir_lowering=False,
        debug=False,
        num_swdge_queues=4,
        dynamic_dma_scratch_size=SCRATCH,
    )

    nfn = nc.declare_dram_parameter("nfn", [NODES_PAD, D], bf16, isOutput=False)
    w = nc.declare_dram_parameter("w", [3 * D, D], bf16, isOutput=False)
    bvec = nc.declare_dram_parameter("bvec", [D, 1], f32, isOutput=False)
    ident = nc.declare_dram_parameter("ident", [D, D], bf16, isOutput=False)
    n_iw = cfg.e_slots // 16
    idxr = nc.declare_dram_parameter("idxr", [D, n_iw], i16, isOutput=False)
    idxc = nc.declare_dram_parameter("idxc", [D, n_iw], i16, isOutput=False)
    eft = nc.declare_dram_parameter("eft", [D, cfg.e_slots], bf16, isOutput=False)
    outp = nc.declare_dram_parameter("out", [D, cfg.e_slots], bf16, isOutput=True)

    with TileContext(nc) as tc:
        with (
            tc.tile_pool(name="const", bufs=1) as cpool,
            tc.tile_pool(name="pps", bufs=2, space="PSUM") as pps,
            tc.tile_pool(name="tps", bufs=2, space="PSUM") as tps,
            tc.tile_pool(name="gat", bufs=6) as gpool,
            tc.tile_pool(name="trs", bufs=4) as trpool,
            tc.tile_pool(name="edg", bufs=3) as epool,
            tc.tile_pool(name="gsb", bufs=3) as gspool,
        ):
            nc.gpsimd.load_library(library_config.mlp)

            w1 = cpool.tile([D, D], bf16, name="w1")
            w2 = cpool.tile([D, D], bf16, name="w2")
            w3 = cpool.tile([D, D], bf16, name="w3")
            bia = cpool.tile([D, 1], f32, name="bia")
            idt = cpool.tile([D, D], bf16, name="idt")
            ira = cpool.tile([D, n_iw], i16, name="ira")
            ica = cpool.tile([D, n_iw], i16, name="ica")

            def emit_consts():
                nc.sync.dma_start(out=w1[:], in_=w[0:D, :])
                nc.sync.dma_start(out=w2[:], in_=w[D : 2 * D, :])
                nc.sync.dma_start(out=w3[:], in_=w[2 * D : 3 * D, :])
                nc.sync.dma_start(out=bia[:], in_=bvec[:, :])
                nc.sync.dma_start(out=idt[:], in_=ident[:, :])
                nc.sync.dma_start(out=ira[:], in_=idxr[:, :])
                nc.sync.dma_start(out=ica[:], in_=idxc[:, :])

            def emit_edges():
                prev_gather = None
                qn = 0
                for sec, off, NI in cfg.groups:
                    rh, ch = sec >> 1, sec & 1
                    i16off = off // 16
                    kmax = NI // D
                    gr = gpool.tile([D, kmax, D], bf16, name="gr")
                    gc = gpool.tile([D, kmax, D], bf16, name="gc")
                    if not no_gather:
                        g1 = nc.gpsimd.dma_gather(
                            gr[:],
                            nfn[rh * H : (rh + 1) * H, :],
                            ira[:, i16off : i16off + NI // 16],
                            NI, NI, D,
                            queue_num=qn % 4,
                            single_packet=True,
                        )
                        g2 = nc.gpsimd.dma_gather(
                            gc[:],
                            nfn[ch * H : (ch + 1) * H, :],
                            ica[:, i16off : i16off + NI // 16],
                            NI, NI, D,
                            queue_num=(qn + 1) % 4,
                            single_packet=True,
                        )
                        qn += 2
                        # keep Pool-engine issue order = program order so
                        # Tile's round-robin DMASW sem lanes stay aligned
                        # with the 4-queue cycle (sems are queue-locked)
                        if prev_gather is not None:
                            add_dep_helper(
                                g1.ins, prev_gather.ins, sync=False,
                                reason="swdge lane/queue alignment",
                            )
                        add_dep_helper(
                            g2.ins, g1.ins, sync=False,
                            reason="swdge lane/queue alignment",
                        )
                        prev_gather = g2
                    et = epool.tile([D, NI], bf16, name="et")
                    if not no_edgedma:
                        nc.sync.dma_start(out=et[:], in_=eft[:, off : off + NI])
                    o_t = epool.tile([D, NI], bf16, name="o_t")
                    if not no_compute:
                        EB = 4  # 128-edge subtiles per PSUM bank (512 edges)
                        for kb in range((kmax + EB - 1) // EB):
                            nb = min(EB, kmax - kb * EB)
                            bsl = slice(kb * EB * D, kb * EB * D + nb * D)
                            ps = pps.tile([D, nb * D], f32, name="ps")
                            if not no_gather:
                                grt = tps.tile([D, nb * D], bf16, name="grt")
                                gct = tps.tile([D, nb * D], bf16, name="gct")
                                for k4 in range(nb):
                                    k = kb * EB + k4
                                    psl = slice(k4 * D, (k4 + 1) * D)
                                    nc.tensor.transpose(
                                        grt[:, psl], gr[:, k, :], idt[:]
                                    )
                                    nc.tensor.transpose(
                                        gct[:, psl], gc[:, k, :], idt[:]
                                    )
                                grs = trpool.tile([D, nb * D], bf16, name="grs")
                                gcs = trpool.tile([D, nb * D], bf16, name="gcs")
                                nc.scalar.copy(grs[:], grt[:])
                                nc.vector.tensor_copy(gcs[:], gct[:])
                                nc.tensor.matmul(
                                    ps[:], w1[:], grs[:], start=True, stop=False
                                )
                                nc.tensor.matmul(
                                    ps[:], w2[:], gcs[:], start=False, stop=False
                                )
                            nc.tensor.matmul(
                                ps[:], w3[:], et[:, bsl],
                                start=no_gather, stop=True,
                            )
                            gate = gspool.tile([D, nb * D], bf16, name="gate")
                            nc.scalar.activation(
                                gate[:], ps[:],
                                mybir.ActivationFunctionType.Sigmoid,
                                bias=bia[:, 0:1],
                            )
                            nc.vector.tensor_mul(
                                o_t[:, bsl], et[:, bsl], gate[:]
                            )
                        nc.sync.dma_start(
                            out=outp[:, off : off + NI], in_=o_t[:]
                        )
                    elif not no_edgedma:
                        nc.sync.dma_start(out=outp[:, off : off + NI], in_=et[:])
                    else:
                        nc.sync.dma_start(
                            out=outp[:, off : off + NI],
                            in_=gr[:].rearrange("p k d -> p (k d)"),
                        )

            def emit_body():
                emit_consts()
                emit_edges()

            if repeat > 1:
                with tc.For_i(0, repeat, 1):
                    emit_body()
            else:
                emit_body()

    nc.finalize()
    return nc


def bucketize(cfg: Cfg, er32, ec32):
    """Sort edges into 4 (row-half, col-half) sections with fixed capacity."""
    sec = (er32 >= H).astype(np.int64) * 2 + (ec32 >= H)
    counts = np.bincount(sec, minlength=4)
    if np.any(counts > np.asarray(cfg.sec_cap)):
        raise ValueError(
            f"section sizes {counts} exceed capacity {cfg.sec_cap}; "
            f"node distribution too skewed for compiled bucket layout"
        )
    # row-sorted within each section: row-gathers become nearly
    # sequential HBM reads (col-gathers stay random)
    order = np.lexsort((er32, sec))
    perm = np.full(cfg.e_slots, -1, dtype=np.int64)
    off = 0
    for s in range(4):
        n = counts[s]
        perm[cfg.sec_off[s] : cfg.sec_off[s] + n] = order[off : off + n]
        off += n
    return perm, counts


def wrap_idx(cfg: Cfg, idx_slot):
    """[e_slots] int16 -> [128, e_slots//16] dma_gather index layout."""
    parts = []
    for s, off, NI in cfg.groups:
        a = idx_slot[off : off + NI].reshape(NI // 16, 16).T  # [16, NI//16]
        parts.append(np.tile(a, (8, 1)))                      # [128, NI//16]
    return np.ascontiguousarray(np.concatenate(parts, axis=1))


def make_in_maps(cfg: Cfg, node_features, edge_index, edge_features, W, b, n_cores):
    nf = np.asarray(node_features, dtype=np.float32)
    nf_pad = np.zeros((NODES_PAD, D), dtype=np.float32)
    nf_pad[: nf.shape[0]] = nf
    nfn = np.ascontiguousarray(nf_pad.astype(BF16))

    w_bf = np.ascontiguousarray(np.asarray(W, dtype=np.float32).astype(BF16))
    bv = np.asarray(b, dtype=np.float32).reshape(D, 1)
    ident = np.eye(D, dtype=BF16)

    ei = np.asarray(edge_index)
    ef = np.asarray(edge_features, dtype=np.float32)

    e_core = ei.shape[1] // n_cores
    in_maps = []
    perms = []
    for i in range(n_cores):
        sl = slice(i * e_core, (i + 1) * e_core)
        er32 = ei[0, sl].astype(np.int32)
        ec32 = ei[1, sl].astype(np.int32)
        ef_bf = ef[sl].astype(BF16)

        perm, _ = bucketize(cfg, er32, ec32)
        filled = perm >= 0
        src = perm[filled]

        er_slot = np.zeros(cfg.e_slots, dtype=np.int32)
        ec_slot = np.zeros(cfg.e_slots, dtype=np.int32)
        er_slot[filled] = er32[src]
        ec_slot[filled] = ec32[src]
        sec_of_slot = np.repeat(np.arange(4), np.asarray(cfg.sec_cap))
        er_slot -= ((sec_of_slot >> 1) * H).astype(np.int32)
        ec_slot -= ((sec_of_slot & 1) * H).astype(np.int32)
        er_slot[~filled] = 0
        ec_slot[~filled] = 0
        assert er_slot.min() >= 0 and er_slot.max() < H
        assert ec_slot.min() >= 0 and ec_slot.max() < H

        ef_slot = np.zeros((cfg.e_slots, D), dtype=BF16)
        ef_slot[filled] = ef_bf[src]

        in_maps.append(
            {
                "nfn": nfn,
                "w": w_bf,
                "bvec": bv,
                "ident": ident,
                "idxr": wrap_idx(cfg, er_slot.astype(np.int16)),
                "idxc": wrap_idx(cfg, ec_slot.astype(np.int16)),
                "eft": np.ascontiguousarray(ef_slot.T),
            }
        )
        perms.append(perm)
    return in_maps, perms


def unpack_out(cfg: Cfg, o, perm, e_core):
    """[D, e_slots] bf16 feat-major slot output -> [e_core, D] f32."""
    slots = np.asarray(o).T.astype(np.float32)  # [e_slots, D]
    res = np.empty((e_core, D), dtype=np.float32)
    filled = perm >= 0
    res[perm[filled]] = slots[filled]
    return res


_CACHE = {}


def derive_cfg(edge_index, n_cores):
    """Pick per-section capacities from the actual index distribution."""
    ei = np.asarray(edge_index)
    e_core = ei.shape[1] // n_cores
    mx = np.zeros(4, dtype=np.int64)
    for i in range(n_cores):
        sl = slice(i * e_core, (i + 1) * e_core)
        sec = (ei[0, sl] >= H).astype(np.int64) * 2 + (ei[1, sl] >= H)
        mx = np.maximum(mx, np.bincount(sec, minlength=4))
    caps = [int(-(-int(c) // 512)) * 512 for c in mx]
    return Cfg(caps)


def kernel(node_features, edge_index, edge_features, W, b):
    from concourse.bass_utils import run_bass_kernel_spmd

    cfg = derive_cfg(edge_index, N_CORES)
    key = cfg.sec_cap
    if key not in _CACHE:
        _CACHE[key] = build_nc(cfg)
    nc = _CACHE[key]

    in_maps, perms = make_in_maps(
        cfg, node_features, edge_index, edge_features, W, b, N_CORES
    )
    res = run_bass_kernel_spmd(nc, in_maps, core_ids=list(range(N_CORES)))
    e_core = np.asarray(edge_index).shape[1] // N_CORES
    outs = [
        unpack_out(cfg, res.results[i]["out"], perms[i], e_core)
        for i in range(N_CORES)
    ]
    return np.concatenate(outs, axis=0)
